# revision 22
# baseline (speedup 1.0000x reference)
"""GIN layer (segment_sum -> combine -> BatchNorm -> ReLU) on 8 TRN2 NeuronCores.

Strategy: dst-shard nodes across 8 cores (6250 nodes each). The edge list is
static, so the host pre-expands each core's gather stream: for every dst block
of 128 slots, a fixed layout of 16 tiles x 128 rows holds (per slot) the
self-term row plus the first 15 in-edge source rows of hn = h*norm (bf16,
zero-padded), followed by a few overflow tiles holding the remaining edges
sorted by slot. The device then:
  1. streams the pre-expanded table with large contiguous HWDGE DMAs
     (no SWDGE gather, no descriptor-generation bottleneck)
  2. segment-sums via TensorE matmuls with 16 FIXED one-hot E tiles
     (slot = 8t + r//16), overflow tiles use a batched is_equal E build
  3. combine: out_pre = psum * norm_dst   (self term pre-divided by norm)
  4. BN stats: S1 via ones-matmul, S2 via accumulated gram-matmul diagonal;
     AllReduce of [1,256] stats; batched affine+ReLU; DMA out.
"""

import sys

sys.path.insert(0, "/opt/trn_rl_repo")

import numpy as np
import ml_dtypes

import concourse.bass as bass
import concourse.bacc as bacc
import concourse.mybir as mybir
import concourse.tile as tile
from concourse.bass_utils import run_bass_kernel_spmd

F32 = mybir.dt.float32
BF16 = mybir.dt.bfloat16
OP = mybir.AluOpType
AF = mybir.ActivationFunctionType

FULL_CFG = dict(
    n_nodes=50000,
    n_edges=800000,
    d=128,
    cores=8,
    blk=128,    # dst slots per psum block
    base=16,    # rows per slot in the fixed base region (1 self + 15 edges)
    grp=4,      # blocks per DMA chunk / combine batch
    bn_eps=1e-5,
)


def _schedule(src, dst, cfg):
    """Host-side edge layout. Returns (sched, per_core dict(eidx, slotb))."""
    n, cores, blkn, base = cfg["n_nodes"], cfg["cores"], cfg["blk"], cfg["base"]
    npc = n // cores
    nblk = -(-npc // blkn)
    spt = blkn // base          # slots per base tile (8)
    nedge_base = base - 1       # edges held in the base region per slot (15)

    core_of = dst // npc
    dloc = dst - core_of * npc

    # per-core sorted edge arrays + overflow counts per block
    per_core = []
    ovf_cnt = np.zeros((cores, nblk), dtype=np.int64)
    for m in range(cores):
        msk = core_of == m
        dl = dloc[msk]
        sr = src[msk].astype(np.int64)
        order = np.argsort(dl, kind="stable")
        dl = dl[order]
        sr = sr[order]
        cnt = np.bincount(dl, minlength=npc)
        starts = np.concatenate([[0], np.cumsum(cnt)])
        rank = np.arange(len(dl)) - starts[dl]
        per_core.append(dict(dl=dl, sr=sr, rank=rank, cnt=cnt))
        ov = np.maximum(cnt - nedge_base, 0)
        ovf_cnt[m] = np.add.reduceat(
            np.pad(ov, (0, nblk * blkn - npc)), np.arange(0, nblk * blkn, blkn)
        )

    T_ovf = -(-ovf_cnt.max(axis=0) // blkn)  # [nblk]
    Tb = base + T_ovf                        # tiles (cols) per block
    gcol = np.concatenate([[0], np.cumsum(Tb)])[:-1]   # starting col of block
    ocol = np.concatenate([[0], np.cumsum(T_ovf)])[:-1]
    nt = int(Tb.sum())
    novf = int(T_ovf.sum())

    # single-block groups at both ends: early pipeline start, and a short
    # dependency chain from the last chunk into the stats AllReduce
    grp = cfg["grp"]
    groups = [[0], [1]]
    b = 2
    while b < nblk - 2:
        groups.append(list(range(b, min(b + grp, nblk - 2))))
        b += grp
    groups += [[nblk - 2], [nblk - 1]]
    ovg_max = max(int(T_ovf[blocks].sum()) for blocks in groups)

    arrs = []
    for m in range(cores):
        pc = per_core[m]
        dl, sr, rank, cnt = pc["dl"], pc["sr"], pc["rank"], pc["cnt"]
        eidx = np.zeros(nt * blkn, dtype=np.int64)
        slotb = np.full((128, max(novf, 1)), 999.0, dtype=np.float32)

        # self rows: slot s entry 0 <- 1 + n + global node
        ln = np.arange(npc)
        b_of = ln // blkn
        s_of = ln % blkn
        flat_self = (gcol[b_of] + s_of // spt) * blkn + (s_of % spt) * base
        eidx[flat_self] = 1 + n + (m * npc + ln)

        # base edges: rank < 15 -> entry j = rank+1
        bm = rank < nedge_base
        lnb = dl[bm]
        bb = lnb // blkn
        sb = lnb % blkn
        flat_b = (gcol[bb] + sb // spt) * blkn + (sb % spt) * base + (rank[bm] + 1)
        eidx[flat_b] = 1 + sr[bm]

        # overflow edges: packed per block in slot order
        om = ~bm
        lno = dl[om]
        bo = lno // blkn
        so = lno % blkn
        sro = sr[om]
        for b in range(nblk):
            sel = bo == b
            k = int(sel.sum())
            if k == 0:
                continue
            pos = np.arange(k)
            flat_o = (gcol[b] + base + pos // blkn) * blkn + pos % blkn
            eidx[flat_o] = 1 + sro[sel]
            slotb[pos % blkn, ocol[b] + pos // blkn] = so[sel]

        arrs.append(dict(eidx=eidx,
                         slotb=slotb.astype(ml_dtypes.bfloat16)))

    # host-built fixed base-E tiles: EB[r, t*128+s] = 1 iff s == spt*t + r//base
    r = np.arange(blkn)
    eb = np.zeros((blkn, base, blkn), np.float32)
    for t in range(base):
        eb[r, t, spt * t + r // base] = 1.0
    ebase = eb.reshape(blkn, base * blkn).astype(ml_dtypes.bfloat16)

    sched = dict(npc=npc, nblk=nblk, nt=nt, novf=novf, Tb=Tb, T_ovf=T_ovf,
                 gcol=gcol, ocol=ocol, groups=groups, ovg_max=ovg_max,
                 ebase=ebase)
    return sched, arrs


def _build(cfg, sched):
    cores, d, blkn, bn_eps = cfg["cores"], cfg["d"], cfg["blk"], cfg["bn_eps"]
    base, grp = cfg["base"], cfg["grp"]
    npc, nblk, nt, novf = sched["npc"], sched["nblk"], sched["nt"], sched["novf"]
    Tb, T_ovf, gcol, ocol = sched["Tb"], sched["T_ovf"], sched["gcol"], sched["ocol"]
    groups, ovg_max = sched["groups"], sched["ovg_max"]
    n_nodes = cfg["n_nodes"]
    repn = max(ovg_max, base)

    nc = bacc.Bacc("TRN2", target_bir_lowering=False, debug=False,
                   num_devices=cores)

    hexp_d = nc.dram_tensor("hexp", [128, nt * blkn], BF16, kind="ExternalInput")
    nrm_d = nc.dram_tensor("nrm", [128, nblk], F32, kind="ExternalInput")
    gb_d = nc.dram_tensor("gb", [1, 2 * d], F32, kind="ExternalInput")
    slotb_d = nc.dram_tensor("slotb", [128, max(novf, 1)], BF16, kind="ExternalInput")
    eb_d = nc.dram_tensor("ebase", [128, base * blkn], BF16, kind="ExternalInput")
    out_d = nc.dram_tensor("out", [nblk, blkn, d], F32, kind="ExternalOutput")

    with tile.TileContext(nc) as tc:
        with (
            tc.tile_pool(name="const", bufs=1) as constp,
            tc.tile_pool(name="meta", bufs=1) as metap,
            tc.tile_pool(name="outpre", bufs=1) as outprep,
            tc.tile_pool(name="spsum", bufs=1, space="PSUM") as spsum,
        ):
            ones_col = constp.tile([128, 1], BF16)
            nc.vector.memset(ones_col[:], 1.0)
            ones_row = constp.tile([1, d], F32)
            nc.vector.memset(ones_row[:], 1.0)
            iota_rep = constp.tile([128, repn, blkn], BF16)
            nc.gpsimd.iota(iota_rep[:], pattern=[[0, repn], [1, blkn]], base=0,
                           channel_multiplier=0, allow_small_or_imprecise_dtypes=True)
            diag_i = constp.tile([128, d], BF16)
            nc.gpsimd.iota(diag_i[:], pattern=[[1, d]], base=0,
                           channel_multiplier=-1, allow_small_or_imprecise_dtypes=True)
            ident = constp.tile([128, d], BF16)
            nc.vector.tensor_scalar(ident[:], diag_i[:], 0.0, None, OP.is_equal)

            # EB leads the sync queue so the first matmul is gated only by
            # the first hexp chunk; small metas ride the scalar HWDGE queue
            EB = constp.tile([128, base, blkn], BF16)
            nc.sync.dma_start(EB[:], eb_d[:])

            nrm_sb = metap.tile([128, nblk], F32)
            gb_sb = metap.tile([1, 2 * d], F32)
            slotb_sb = metap.tile([128, max(novf, 1)], BF16)
            nc.scalar.dma_start(slotb_sb[:], slotb_d[:])
            nc.scalar.dma_start(nrm_sb[:], nrm_d[:])
            nc.scalar.dma_start(gb_sb[:], gb_d[:])

            # warm the CC stream early so the real AllReduce at the end only
            # pays per-op latency, and preload the Sqrt ACT table
            warm_sb = metap.tile([1, 8], F32)
            nc.vector.memset(warm_sb[:], 1.0)
            warm_sq = metap.tile([1, 8], F32)
            nc.scalar.activation(warm_sq[:], warm_sb[:], AF.Sqrt)

            outpre = outprep.tile([128, nblk, d], BF16)
            s1_ps = spsum.tile([1, d], F32)
            gram_ps = spsum.tile([128, d], F32)

            with tc.tile_pool(name="dram0", bufs=1, space="DRAM") as dramp0:
                warm_in = dramp0.tile([1, 8], F32)
                warm_out = dramp0.tile([1, 8], F32)
                nc.scalar.dma_start(warm_in[:], warm_sq[:])
                nc.gpsimd.collective_compute(
                    "AllReduce", OP.add,
                    replica_groups=[list(range(cores))],
                    ins=[warm_in.opt()], outs=[warm_out.opt()],
                )

            with (
                tc.tile_pool(name="gpool", bufs=4) as gpool,
                tc.tile_pool(name="eov", bufs=2) as eovp,
                tc.tile_pool(name="npsum", bufs=2, space="PSUM") as npsum,
            ):
                for g, blocks in enumerate(groups):
                    b0 = blocks[0]
                    nb = len(blocks)
                    c0 = int(gcol[b0])
                    gcols = int(Tb[blocks].sum())
                    gt = gpool.tile([128, gcols * d], BF16, tag="g")
                    eng = nc.sync if g % 2 == 0 else nc.scalar
                    eng.dma_start(gt[:], hexp_d[:, c0 * d : (c0 + gcols) * d])

                    ovg = int(T_ovf[blocks].sum())
                    if ovg > 0:
                        o0 = int(ocol[b0])
                        Eov = eovp.tile([128, ovg, blkn], BF16, tag="e")
                        nc.vector.tensor_tensor(
                            Eov[:], iota_rep[:, :ovg, :],
                            slotb_sb[:, o0 : o0 + ovg].to_broadcast([128, ovg, blkn]),
                            OP.is_equal)

                    ps_g = npsum.tile([128, grp, d], F32, tag="ps")
                    for bi, b in enumerate(blocks):
                        ntile_b = int(Tb[b])
                        cloc = int(gcol[b]) - c0
                        oloc = int(ocol[b] - ocol[b0]) if ovg > 0 else 0
                        pssl = ps_g[:, bi, :]
                        for k in range(ntile_b):
                            if k < base:
                                E = EB[:, k, :]
                            else:
                                E = Eov[:, oloc + (k - base), :]
                            rhs = gt[:, (cloc + k) * d : (cloc + k + 1) * d]
                            nc.tensor.matmul(pssl, E, rhs,
                                             start=(k == 0), stop=(k == ntile_b - 1),
                                             skip_group_check=True)

                    # combine: out_pre = psum * norm_dst (batched over the group)
                    nc.vector.tensor_tensor(
                        outpre[:, b0 : b0 + nb, :], ps_g[:, :nb, :],
                        nrm_sb[:, b0 : b0 + nb].to_broadcast([128, nb, blkn]),
                        OP.mult)

                    # BN stats accumulation
                    for b in blocks:
                        op_sl = outpre[:, b, :]
                        nc.tensor.matmul(s1_ps[:], ones_col[:], op_sl,
                                         start=(b == 0), stop=(b == nblk - 1),
                                         skip_group_check=True)
                        nc.tensor.matmul(gram_ps[:], op_sl, op_sl,
                                         start=(b == 0), stop=(b == nblk - 1),
                                         skip_group_check=True)

            # ---- BatchNorm tail ----
            with (
                tc.tile_pool(name="bn", bufs=1) as bnp,
                tc.tile_pool(name="bnps", bufs=1, space="PSUM") as bnps,
                tc.tile_pool(name="dram", bufs=1, space="DRAM") as dramp,
                tc.tile_pool(name="fin", bufs=3) as finp,
            ):
                masked = bnp.tile([128, d], BF16)
                nc.vector.tensor_tensor(masked[:], gram_ps[:], ident[:], OP.mult)
                s2_ps = bnps.tile([1, d], F32)
                nc.tensor.matmul(s2_ps[:], ones_col[:], masked[:])
                stats = bnp.tile([1, 2 * d], F32)
                nc.vector.tensor_copy(stats[:, :d], s1_ps[:])
                nc.vector.tensor_copy(stats[:, d:], s2_ps[:])

                cc_in = dramp.tile([1, 2 * d], F32)
                cc_out = dramp.tile([1, 2 * d], F32)
                nc.sync.dma_start(cc_in[:], stats[:])
                nc.gpsimd.collective_compute(
                    "AllReduce", OP.add,
                    replica_groups=[list(range(cores))],
                    ins=[cc_in.opt()], outs=[cc_out.opt()],
                )
                gstats = bnp.tile([1, 2 * d], F32)
                nc.sync.dma_start(gstats[:], cc_out[:])

                inv_n = 1.0 / float(n_nodes)
                scaled = bnp.tile([1, 2 * d], F32)
                nc.vector.tensor_scalar(scaled[:], gstats[:], inv_n, None, OP.mult)
                mu = scaled[:, :d]
                musq = bnp.tile([1, d], F32)
                nc.vector.tensor_tensor(musq[:], mu, mu, OP.mult)
                var = bnp.tile([1, d], F32)
                nc.vector.tensor_tensor(var[:], scaled[:, d:], musq[:], OP.subtract)
                epsb = bnp.tile([1, 1], F32)
                nc.vector.memset(epsb[:], float(bn_eps))
                std = bnp.tile([1, d], F32)
                nc.scalar.activation(std[:], var[:], AF.Sqrt, bias=epsb[:])
                rstd = bnp.tile([1, d], F32)
                nc.vector.reciprocal(rstd[:], std[:])
                gvec = bnp.tile([1, d], F32)
                nc.vector.tensor_tensor(gvec[:], gb_sb[:, :d], rstd[:], OP.mult)
                mg = bnp.tile([1, d], F32)
                nc.vector.tensor_tensor(mg[:], mu, gvec[:], OP.mult)
                bvec = bnp.tile([1, d], F32)
                nc.vector.tensor_tensor(bvec[:], gb_sb[:, d:], mg[:], OP.subtract)

                gbvec = bnp.tile([1, 2 * d], F32)
                nc.vector.tensor_copy(gbvec[:, :d], gvec[:])
                nc.vector.tensor_copy(gbvec[:, d:], bvec[:])
                gb_ps = bnps.tile([128, 2 * d], F32)
                nc.tensor.matmul(gb_ps[:], ones_row[:], gbvec[:])
                gb_bc = bnp.tile([128, 2 * d], BF16)
                nc.vector.tensor_copy(gb_bc[:], gb_ps[:])
                g_bc = gb_bc
                b_bc_ap = gb_bc[:, d:]

                CH = 13
                for c0b in range(0, nblk, CH):
                    cn = min(CH, nblk - c0b)
                    t1 = finp.tile([128, CH, d], BF16, tag="t1")
                    nc.vector.tensor_tensor(
                        t1[:, :cn, :], outpre[:, c0b : c0b + cn, :],
                        gb_bc[:, :d][:, None, :].to_broadcast([128, cn, d]), OP.mult)
                    t2 = finp.tile([128, CH, d], BF16, tag="t2")
                    nc.vector.tensor_tensor(
                        t2[:, :cn, :], t1[:, :cn, :],
                        gb_bc[:, d:][:, None, :].to_broadcast([128, cn, d]), OP.add)
                    fin = finp.tile([128, CH, d], F32, tag="fin")
                    nc.scalar.activation(fin[:, :cn, :], t2[:, :cn, :], AF.Relu)
                    ov = out_d[c0b : c0b + cn, :, :].transpose([1, 0, 2])
                    nc.sync.dma_start(ov, fin[:, :cn, :])

    nc.compile()
    return nc


_CACHE = {}


def _get_compiled(cfg, src, dst):
    key = (cfg["n_nodes"], cfg["n_edges"], cfg["blk"], cfg["grp"], cfg["base"],
           hash(src.tobytes()), hash(dst.tobytes()))
    if key not in _CACHE:
        sched, arrs = _schedule(src, dst, cfg)
        nc = _build(cfg, sched)
        _CACHE[key] = (nc, sched, arrs)
    return _CACHE[key]


def run(h, norm, eps, gamma, beta, src, dst, cfg=None, trace=False):
    cfg = cfg or FULL_CFG
    h = np.asarray(h, np.float32)
    norm = np.asarray(norm, np.float32)
    src = np.asarray(src, np.int32)
    dst = np.asarray(dst, np.int32)
    eps_val = float(np.asarray(eps).reshape(-1)[0])
    gamma = np.asarray(gamma, np.float32).reshape(1, -1)
    beta = np.asarray(beta, np.float32).reshape(1, -1)

    nc, sched, arrs = _get_compiled(cfg, src, dst)

    n, cores, d, blkn = cfg["n_nodes"], cfg["cores"], cfg["d"], cfg["blk"]
    npc, nblk, nt = sched["npc"], sched["nblk"], sched["nt"]

    nrm_col = norm.reshape(-1, 1)
    hn = (h * nrm_col).astype(ml_dtypes.bfloat16)
    selfp = ((1.0 + eps_val) * h * nrm_col).astype(ml_dtypes.bfloat16)
    S = np.concatenate(
        [np.zeros((1, d), ml_dtypes.bfloat16), hn, selfp], axis=0)
    gb = np.concatenate([gamma, beta], axis=1).astype(np.float32)

    in_maps = []
    for m in range(cores):
        vals = S[arrs[m]["eidx"]]                       # [nt*128, d] bf16
        hexp = np.ascontiguousarray(
            vals.reshape(nt, blkn, d).transpose(1, 0, 2).reshape(128, nt * d))
        nr = np.zeros((128, nblk), np.float32)
        nr_flat = np.zeros(nblk * blkn, np.float32)
        nr_flat[:npc] = norm.reshape(-1)[m * npc : (m + 1) * npc]
        nr[:, :] = nr_flat.reshape(nblk, blkn).T
        in_maps.append(
            dict(hexp=hexp, nrm=nr, gb=gb,
                 slotb=arrs[m]["slotb"], ebase=sched["ebase"])
        )

    res = run_bass_kernel_spmd(nc, in_maps, list(range(cores)), trace=trace)
    out = np.concatenate(
        [res.results[m]["out"].reshape(nblk * blkn, d)[:npc] for m in range(cores)],
        axis=0)
    return out.astype(np.float32), res


def kernel(h, norm, eps, gamma, beta, src, dst):
    out, _ = run(h, norm, eps, gamma, beta, src, dst)
    return out


# revision 23
# speedup vs baseline: 1.0848x; 1.0848x over previous
"""GIN layer (segment_sum -> combine -> BatchNorm -> ReLU) on 8 TRN2 NeuronCores.

Strategy: dst-shard nodes across 8 cores (6250 nodes each). The edge list is
static, so the host pre-expands each core's gather stream: for every dst block
of 128 slots, a fixed layout of 16 tiles x 128 rows holds (per slot) the
self-term row plus the first 15 in-edge source rows of hn = h*norm (bf16,
zero-padded), followed by a few overflow tiles holding the remaining edges
sorted by slot. The device then:
  1. streams the pre-expanded table with large contiguous HWDGE DMAs
     (no SWDGE gather, no descriptor-generation bottleneck)
  2. segment-sums via TensorE matmuls with 16 FIXED one-hot E tiles
     (slot = 8t + r//16), overflow tiles use a batched is_equal E build
  3. combine: out_pre = psum * norm_dst   (self term pre-divided by norm)
  4. BN stats: S1 via ones-matmul, S2 via accumulated gram-matmul diagonal;
     AllReduce of [1,256] stats; batched affine+ReLU; DMA out.
"""

import sys

sys.path.insert(0, "/opt/trn_rl_repo")

import numpy as np
import ml_dtypes

import concourse.bass as bass
import concourse.bacc as bacc
import concourse.mybir as mybir
import concourse.tile as tile
from concourse.bass_utils import run_bass_kernel_spmd

F32 = mybir.dt.float32
BF16 = mybir.dt.bfloat16
OP = mybir.AluOpType
AF = mybir.ActivationFunctionType

FULL_CFG = dict(
    n_nodes=50000,
    n_edges=800000,
    d=128,
    cores=8,
    blk=128,    # dst slots per psum block
    base=16,    # rows per slot in the fixed base region (1 self + 15 edges)
    grp=4,      # blocks per DMA chunk / combine batch
    bn_eps=1e-5,
)


def _schedule(src, dst, cfg):
    """Host-side edge layout. Returns (sched, per_core dict(eidx, slotb))."""
    n, cores, blkn, base = cfg["n_nodes"], cfg["cores"], cfg["blk"], cfg["base"]
    npc = n // cores
    nblk = -(-npc // blkn)
    spt = blkn // base          # slots per base tile (8)
    nedge_base = base - 1       # edges held in the base region per slot (15)

    core_of = dst // npc
    dloc = dst - core_of * npc

    # per-core sorted edge arrays + overflow counts per block
    per_core = []
    ovf_cnt = np.zeros((cores, nblk), dtype=np.int64)
    for m in range(cores):
        msk = core_of == m
        dl = dloc[msk]
        sr = src[msk].astype(np.int64)
        order = np.argsort(dl, kind="stable")
        dl = dl[order]
        sr = sr[order]
        cnt = np.bincount(dl, minlength=npc)
        starts = np.concatenate([[0], np.cumsum(cnt)])
        rank = np.arange(len(dl)) - starts[dl]
        per_core.append(dict(dl=dl, sr=sr, rank=rank, cnt=cnt))
        ov = np.maximum(cnt - nedge_base, 0)
        ovf_cnt[m] = np.add.reduceat(
            np.pad(ov, (0, nblk * blkn - npc)), np.arange(0, nblk * blkn, blkn)
        )

    T_ovf = -(-ovf_cnt.max(axis=0) // blkn)  # [nblk]
    Tb = base + T_ovf                        # tiles (cols) per block
    gcol = np.concatenate([[0], np.cumsum(Tb)])[:-1]   # starting col of block
    ocol = np.concatenate([[0], np.cumsum(T_ovf)])[:-1]
    nt = int(Tb.sum())
    novf = int(T_ovf.sum())

    # single-block groups at both ends: early pipeline start, and a short
    # dependency chain from the last chunk into the stats AllReduce
    grp = cfg["grp"]
    groups = [[0], [1]]
    b = 2
    while b < nblk - 2:
        groups.append(list(range(b, min(b + grp, nblk - 2))))
        b += grp
    groups += [[nblk - 2], [nblk - 1]]
    ovg_max = max(int(T_ovf[blocks].sum()) for blocks in groups)

    arrs = []
    for m in range(cores):
        pc = per_core[m]
        dl, sr, rank, cnt = pc["dl"], pc["sr"], pc["rank"], pc["cnt"]
        eidx = np.zeros(nt * blkn, dtype=np.int64)
        slotb = np.full((128, max(novf, 1)), 999.0, dtype=np.float32)

        # self rows: slot s entry 0 <- 1 + n + global node
        ln = np.arange(npc)
        b_of = ln // blkn
        s_of = ln % blkn
        flat_self = (gcol[b_of] + s_of // spt) * blkn + (s_of % spt) * base
        eidx[flat_self] = 1 + n + (m * npc + ln)

        # base edges: rank < 15 -> entry j = rank+1
        bm = rank < nedge_base
        lnb = dl[bm]
        bb = lnb // blkn
        sb = lnb % blkn
        flat_b = (gcol[bb] + sb // spt) * blkn + (sb % spt) * base + (rank[bm] + 1)
        eidx[flat_b] = 1 + sr[bm]

        # overflow edges: packed per block in slot order
        om = ~bm
        lno = dl[om]
        bo = lno // blkn
        so = lno % blkn
        sro = sr[om]
        for b in range(nblk):
            sel = bo == b
            k = int(sel.sum())
            if k == 0:
                continue
            pos = np.arange(k)
            flat_o = (gcol[b] + base + pos // blkn) * blkn + pos % blkn
            eidx[flat_o] = 1 + sro[sel]
            slotb[pos % blkn, ocol[b] + pos // blkn] = so[sel]

        arrs.append(dict(eidx=eidx,
                         slotb=slotb.astype(ml_dtypes.bfloat16)))

    # host-built fixed base-E tiles: EB[r, t*128+s] = 1 iff s == spt*t + r//base
    r = np.arange(blkn)
    eb = np.zeros((blkn, base, blkn), np.float32)
    for t in range(base):
        eb[r, t, spt * t + r // base] = 1.0
    ebase = eb.reshape(blkn, base * blkn).astype(ml_dtypes.bfloat16)

    sched = dict(npc=npc, nblk=nblk, nt=nt, novf=novf, Tb=Tb, T_ovf=T_ovf,
                 gcol=gcol, ocol=ocol, groups=groups, ovg_max=ovg_max,
                 ebase=ebase)
    return sched, arrs


def _build(cfg, sched):
    cores, d, blkn, bn_eps = cfg["cores"], cfg["d"], cfg["blk"], cfg["bn_eps"]
    base, grp = cfg["base"], cfg["grp"]
    npc, nblk, nt, novf = sched["npc"], sched["nblk"], sched["nt"], sched["novf"]
    Tb, T_ovf, gcol, ocol = sched["Tb"], sched["T_ovf"], sched["gcol"], sched["ocol"]
    groups, ovg_max = sched["groups"], sched["ovg_max"]
    n_nodes = cfg["n_nodes"]
    repn = max(ovg_max, base)

    nc = bacc.Bacc("TRN2", target_bir_lowering=False, debug=False,
                   num_devices=cores)

    hexp_d = nc.dram_tensor("hexp", [128, nt * blkn], BF16, kind="ExternalInput")
    nrm_d = nc.dram_tensor("nrm", [128, nblk], F32, kind="ExternalInput")
    gb_d = nc.dram_tensor("gb", [1, 2 * d], F32, kind="ExternalInput")
    slotb_d = nc.dram_tensor("slotb", [128, max(novf, 1)], BF16, kind="ExternalInput")
    eb_d = nc.dram_tensor("ebase", [128, base * blkn], BF16, kind="ExternalInput")
    out_d = nc.dram_tensor("out", [nblk, blkn, d], F32, kind="ExternalOutput")

    with tile.TileContext(nc) as tc:
        with (
            tc.tile_pool(name="const", bufs=1) as constp,
            tc.tile_pool(name="meta", bufs=1) as metap,
            tc.tile_pool(name="outpre", bufs=1) as outprep,
            tc.tile_pool(name="spsum", bufs=1, space="PSUM") as spsum,
        ):
            ones_col = constp.tile([128, 1], BF16)
            nc.vector.memset(ones_col[:], 1.0)
            ones_row = constp.tile([1, d], F32)
            nc.vector.memset(ones_row[:], 1.0)
            iota_rep = constp.tile([128, repn, blkn], BF16)
            nc.gpsimd.iota(iota_rep[:], pattern=[[0, repn], [1, blkn]], base=0,
                           channel_multiplier=0, allow_small_or_imprecise_dtypes=True)
            diag_i = constp.tile([128, d], BF16)
            nc.gpsimd.iota(diag_i[:], pattern=[[1, d]], base=0,
                           channel_multiplier=-1, allow_small_or_imprecise_dtypes=True)
            ident = constp.tile([128, d], BF16)
            nc.vector.tensor_scalar(ident[:], diag_i[:], 0.0, None, OP.is_equal)

            # EB leads the sync queue so the first matmul is gated only by
            # the first hexp chunk; small metas ride the scalar HWDGE queue
            EB = constp.tile([128, base, blkn], BF16)
            nc.sync.dma_start(EB[:], eb_d[:])

            nrm_sb = metap.tile([128, nblk], F32)
            gb_sb = metap.tile([1, 2 * d], F32)
            slotb_sb = metap.tile([128, max(novf, 1)], BF16)
            nc.sync.dma_start(slotb_sb[:], slotb_d[:])
            nc.sync.dma_start(nrm_sb[:], nrm_d[:])
            nc.sync.dma_start(gb_sb[:], gb_d[:])

            # warm the CC stream early so the real AllReduce at the end only
            # pays per-op latency, and preload the Sqrt ACT table
            warm_sb = metap.tile([1, 8], F32)
            nc.vector.memset(warm_sb[:], 1.0)
            warm_sq = metap.tile([1, 8], F32)
            nc.scalar.activation(warm_sq[:], warm_sb[:], AF.Sqrt)

            outpre = outprep.tile([128, nblk, d], BF16)
            s1_ps = spsum.tile([1, d], F32)
            gram_ps = spsum.tile([128, d], F32)

            with tc.tile_pool(name="dram0", bufs=1, space="DRAM") as dramp0:
                warm_in = dramp0.tile([1, 8], F32)
                warm_out = dramp0.tile([1, 8], F32)
                nc.scalar.dma_start(warm_in[:], warm_sq[:])
                nc.gpsimd.collective_compute(
                    "AllReduce", OP.add,
                    replica_groups=[list(range(cores))],
                    ins=[warm_in.opt()], outs=[warm_out.opt()],
                )

            with (
                tc.tile_pool(name="gpool", bufs=4) as gpool,
                tc.tile_pool(name="eov", bufs=2) as eovp,
                tc.tile_pool(name="npsum", bufs=2, space="PSUM") as npsum,
            ):
                for g, blocks in enumerate(groups):
                    b0 = blocks[0]
                    nb = len(blocks)
                    c0 = int(gcol[b0])
                    gcols = int(Tb[blocks].sum())
                    gt = gpool.tile([128, gcols * d], BF16, tag="g")
                    eng = nc.scalar if g % 2 == 0 else nc.sync
                    eng.dma_start(gt[:], hexp_d[:, c0 * d : (c0 + gcols) * d])

                    ovg = int(T_ovf[blocks].sum())
                    if ovg > 0:
                        o0 = int(ocol[b0])
                        Eov = eovp.tile([128, ovg, blkn], BF16, tag="e")
                        nc.vector.tensor_tensor(
                            Eov[:], iota_rep[:, :ovg, :],
                            slotb_sb[:, o0 : o0 + ovg].to_broadcast([128, ovg, blkn]),
                            OP.is_equal)

                    ps_g = npsum.tile([128, grp, d], F32, tag="ps")
                    for bi, b in enumerate(blocks):
                        ntile_b = int(Tb[b])
                        cloc = int(gcol[b]) - c0
                        oloc = int(ocol[b] - ocol[b0]) if ovg > 0 else 0
                        pssl = ps_g[:, bi, :]
                        for k in range(ntile_b):
                            if k < base:
                                E = EB[:, k, :]
                            else:
                                E = Eov[:, oloc + (k - base), :]
                            rhs = gt[:, (cloc + k) * d : (cloc + k + 1) * d]
                            nc.tensor.matmul(pssl, E, rhs,
                                             start=(k == 0), stop=(k == ntile_b - 1),
                                             skip_group_check=True)

                    # combine: out_pre = psum * norm_dst (batched over the group)
                    nc.vector.tensor_tensor(
                        outpre[:, b0 : b0 + nb, :], ps_g[:, :nb, :],
                        nrm_sb[:, b0 : b0 + nb].to_broadcast([128, nb, blkn]),
                        OP.mult)

                    # BN stats accumulation
                    for b in blocks:
                        op_sl = outpre[:, b, :]
                        nc.tensor.matmul(s1_ps[:], ones_col[:], op_sl,
                                         start=(b == 0), stop=(b == nblk - 1),
                                         skip_group_check=True)
                        nc.tensor.matmul(gram_ps[:], op_sl, op_sl,
                                         start=(b == 0), stop=(b == nblk - 1),
                                         skip_group_check=True)

            # ---- BatchNorm tail ----
            with (
                tc.tile_pool(name="bn", bufs=1) as bnp,
                tc.tile_pool(name="bnps", bufs=1, space="PSUM") as bnps,
                tc.tile_pool(name="dram", bufs=1, space="DRAM") as dramp,
                tc.tile_pool(name="fin", bufs=3) as finp,
            ):
                masked = bnp.tile([128, d], BF16)
                nc.vector.tensor_tensor(masked[:], gram_ps[:], ident[:], OP.mult)
                s2_ps = bnps.tile([1, d], F32)
                nc.tensor.matmul(s2_ps[:], ones_col[:], masked[:])
                stats = bnp.tile([1, 2 * d], F32)
                nc.vector.tensor_copy(stats[:, :d], s1_ps[:])
                nc.vector.tensor_copy(stats[:, d:], s2_ps[:])

                cc_in = dramp.tile([1, 2 * d], F32)
                cc_out = dramp.tile([1, 2 * d], F32)
                nc.sync.dma_start(cc_in[:], stats[:])
                nc.gpsimd.collective_compute(
                    "AllReduce", OP.add,
                    replica_groups=[list(range(cores))],
                    ins=[cc_in.opt()], outs=[cc_out.opt()],
                )
                gstats = bnp.tile([1, 2 * d], F32)
                nc.sync.dma_start(gstats[:], cc_out[:])

                inv_n = 1.0 / float(n_nodes)
                scaled = bnp.tile([1, 2 * d], F32)
                nc.vector.tensor_scalar(scaled[:], gstats[:], inv_n, None, OP.mult)
                mu = scaled[:, :d]
                musq = bnp.tile([1, d], F32)
                nc.vector.tensor_tensor(musq[:], mu, mu, OP.mult)
                var = bnp.tile([1, d], F32)
                nc.vector.tensor_tensor(var[:], scaled[:, d:], musq[:], OP.subtract)
                epsb = bnp.tile([1, 1], F32)
                nc.vector.memset(epsb[:], float(bn_eps))
                std = bnp.tile([1, d], F32)
                nc.scalar.activation(std[:], var[:], AF.Sqrt, bias=epsb[:])
                rstd = bnp.tile([1, d], F32)
                nc.vector.reciprocal(rstd[:], std[:])
                gvec = bnp.tile([1, d], F32)
                nc.vector.tensor_tensor(gvec[:], gb_sb[:, :d], rstd[:], OP.mult)
                mg = bnp.tile([1, d], F32)
                nc.vector.tensor_tensor(mg[:], mu, gvec[:], OP.mult)
                bvec = bnp.tile([1, d], F32)
                nc.vector.tensor_tensor(bvec[:], gb_sb[:, d:], mg[:], OP.subtract)

                gbvec = bnp.tile([1, 2 * d], F32)
                nc.vector.tensor_copy(gbvec[:, :d], gvec[:])
                nc.vector.tensor_copy(gbvec[:, d:], bvec[:])
                gb_ps = bnps.tile([128, 2 * d], F32)
                nc.tensor.matmul(gb_ps[:], ones_row[:], gbvec[:])
                gb_bc = bnp.tile([128, 2 * d], BF16)
                nc.vector.tensor_copy(gb_bc[:], gb_ps[:])
                g_bc = gb_bc
                b_bc_ap = gb_bc[:, d:]

                CH = 13
                for c0b in range(0, nblk, CH):
                    cn = min(CH, nblk - c0b)
                    t1 = finp.tile([128, CH, d], BF16, tag="t1")
                    nc.vector.tensor_tensor(
                        t1[:, :cn, :], outpre[:, c0b : c0b + cn, :],
                        gb_bc[:, :d][:, None, :].to_broadcast([128, cn, d]), OP.mult)
                    t2 = finp.tile([128, CH, d], BF16, tag="t2")
                    nc.vector.tensor_tensor(
                        t2[:, :cn, :], t1[:, :cn, :],
                        gb_bc[:, d:][:, None, :].to_broadcast([128, cn, d]), OP.add)
                    fin = finp.tile([128, CH, d], F32, tag="fin")
                    nc.scalar.activation(fin[:, :cn, :], t2[:, :cn, :], AF.Relu)
                    ov = out_d[c0b : c0b + cn, :, :].transpose([1, 0, 2])
                    nc.sync.dma_start(ov, fin[:, :cn, :])

    nc.compile()
    return nc


_CACHE = {}


def _get_compiled(cfg, src, dst):
    key = (cfg["n_nodes"], cfg["n_edges"], cfg["blk"], cfg["grp"], cfg["base"],
           hash(src.tobytes()), hash(dst.tobytes()))
    if key not in _CACHE:
        sched, arrs = _schedule(src, dst, cfg)
        nc = _build(cfg, sched)
        _CACHE[key] = (nc, sched, arrs)
    return _CACHE[key]


def run(h, norm, eps, gamma, beta, src, dst, cfg=None, trace=False):
    cfg = cfg or FULL_CFG
    h = np.asarray(h, np.float32)
    norm = np.asarray(norm, np.float32)
    src = np.asarray(src, np.int32)
    dst = np.asarray(dst, np.int32)
    eps_val = float(np.asarray(eps).reshape(-1)[0])
    gamma = np.asarray(gamma, np.float32).reshape(1, -1)
    beta = np.asarray(beta, np.float32).reshape(1, -1)

    nc, sched, arrs = _get_compiled(cfg, src, dst)

    n, cores, d, blkn = cfg["n_nodes"], cfg["cores"], cfg["d"], cfg["blk"]
    npc, nblk, nt = sched["npc"], sched["nblk"], sched["nt"]

    nrm_col = norm.reshape(-1, 1)
    hn = (h * nrm_col).astype(ml_dtypes.bfloat16)
    selfp = ((1.0 + eps_val) * h * nrm_col).astype(ml_dtypes.bfloat16)
    S = np.concatenate(
        [np.zeros((1, d), ml_dtypes.bfloat16), hn, selfp], axis=0)
    gb = np.concatenate([gamma, beta], axis=1).astype(np.float32)

    in_maps = []
    for m in range(cores):
        vals = S[arrs[m]["eidx"]]                       # [nt*128, d] bf16
        hexp = np.ascontiguousarray(
            vals.reshape(nt, blkn, d).transpose(1, 0, 2).reshape(128, nt * d))
        nr = np.zeros((128, nblk), np.float32)
        nr_flat = np.zeros(nblk * blkn, np.float32)
        nr_flat[:npc] = norm.reshape(-1)[m * npc : (m + 1) * npc]
        nr[:, :] = nr_flat.reshape(nblk, blkn).T
        in_maps.append(
            dict(hexp=hexp, nrm=nr, gb=gb,
                 slotb=arrs[m]["slotb"], ebase=sched["ebase"])
        )

    res = run_bass_kernel_spmd(nc, in_maps, list(range(cores)), trace=trace)
    out = np.concatenate(
        [res.results[m]["out"].reshape(nblk * blkn, d)[:npc] for m in range(cores)],
        axis=0)
    return out.astype(np.float32), res


def kernel(h, norm, eps, gamma, beta, src, dst):
    out, _ = run(h, norm, eps, gamma, beta, src, dst)
    return out
